# revision 16
# baseline (speedup 1.0000x reference)
# Trainium2 Bass kernel for nn_Encoder_24575802868358 (char-CNN encoder).
#
# Computation (per word): char-embedding lookup -> depthwise grouped conv
# (30 groups x 30 multipliers, k=3, VALID) -> max-over-time pool -> concat
# with a GloVe embedding lookup.  Output [64, 128, 1000] f32.
#
# Strategy (data-parallel over the 8 cores; each core owns 1024 words):
#   * char "gather" via one-hot matmul: char ids are broadcast-DMA'd across
#     102 partitions, DVE is_equal against a per-partition iota builds the
#     one-hot OHT[v, pos] (4x bf16 mode), and a PE matmul with the char
#     table as stationary weights produces X_T[c, pos] directly in the
#     transposed layout the conv needs.  (A per-position indirect gather
#     costs ~300us of GPSIMD descriptor generation - measured - while this
#     path is ~15us PE + ~11us DVE.)
#   * im2col: ScalarE evicts X_T from PSUM to SBUF (bf16), two SBUF->SBUF
#     DMA shifted copies build XT3[30k+c, pos] = X_T[c, pos+k].
#   * conv as one K=90 matmul per (channel-tile, 13-word position tile):
#     the block-diagonal fused weight W2[(k*30+c), o] = w[o,k] * [c==o//30]
#     turns the depthwise conv into dense matmuls; PSUM gets [ch, word, t].
#   * max-over-time: DVE tensor_reduce(max) over strided [ch, 2, 13, 38]
#     PSUM views; bias added on ScalarE afterwards (max(y)+b == max(y+b)).
#   * GloVe rows: indirect DMA gather (int32 indices) -> output.
# Host side only reshapes/concatenates per-core outputs.

import numpy as np
import ml_dtypes

import concourse.bass as bass
import concourse.mybir as mybir
from concourse import bacc
from concourse.tile import TileContext
from concourse.bass_utils import run_bass_kernel_spmd

# ---------------------------------------------------------------- constants
B, S, WLEN = 64, 128, 40
CHAR_EMB = 30
N_FILT = 30
KSIZE = 3
WORD_EMB = 100
N_CHARS = 102
VOCAB = 400002
NCH = CHAR_EMB * N_FILT          # 900 conv output channels
TVALID = WLEN - KSIZE + 1        # 38 valid conv positions

NCORES = 8
WORDS = B * S                    # 8192
WPC = WORDS // NCORES            # 1024 words per core
NCHUNK = 4
CHUNK_W = WPC // NCHUNK          # 256 words per chunk
POS_C = CHUNK_W * WLEN           # 10240 positions per chunk
NI = 10368                       # padded positions per chunk (>= POS_C+2)

CHT = [113, 113, 113, 113, 112, 112, 112, 112]   # channel tile sizes
CHOFF = np.concatenate([[0], np.cumsum(CHT)])    # offsets into 900

# conv matmuls: 19 x 13-word (N=494) + 1 x 9-word (N=342) per chunk/chtile,
# paired two per [128, 1024] PSUM tile (at free offsets 0 and 512).
MM_WORDS = [13] * 19 + [9]

_CACHE = {}


# ---------------------------------------------------------------- program
def build_program():
    nc = bacc.Bacc("TRN2", target_bir_lowering=False)
    f32, bf16 = mybir.dt.float32, mybir.dt.bfloat16

    ctab = nc.declare_dram_parameter("ctab", [N_CHARS, CHAR_EMB], bf16,
                                     isOutput=False)
    w2 = nc.declare_dram_parameter("w2", [90, NCH], bf16, isOutput=False)
    biasp = nc.declare_dram_parameter("biasp", [128, 8], f32, isOutput=False)
    iota = nc.declare_dram_parameter("iota", [128, 1], f32, isOutput=False)
    cids = nc.declare_dram_parameter("cids", [NCHUNK, NI], bf16, isOutput=False)
    gidx = nc.declare_dram_parameter("gidx", [128, WPC // 128], mybir.dt.int32,
                                     isOutput=False)
    glove = nc.declare_dram_parameter("glove", [VOCAB, WORD_EMB], f32,
                                     isOutput=False)
    pooled_d = nc.declare_dram_parameter("pooled", [NCH, WPC], f32, isOutput=True)
    gout_d = nc.declare_dram_parameter("gout", [WPC, WORD_EMB], f32, isOutput=True)

    with TileContext(nc) as tc:
        with (
            tc.tile_pool(name="const", bufs=1) as cpool,
            tc.tile_pool(name="work", bufs=2) as wpool,
            tc.tile_pool(name="small", bufs=3) as spool,
            tc.tile_pool(name="cps", bufs=3, space="PSUM") as cpspool,
            tc.tile_pool(name="xps", bufs=2, space="PSUM") as xpspool,
        ):
            w2_s = cpool.tile([90, NCH], bf16, name="w2_s")
            nc.sync.dma_start(out=w2_s[:], in_=w2[:])
            bias_s = cpool.tile([128, 8], f32, name="bias_s")
            nc.sync.dma_start(out=bias_s[:], in_=biasp[:])
            iota_s = cpool.tile([128, 1], f32, name="iota_s")
            nc.sync.dma_start(out=iota_s[:], in_=iota[:])
            ctab_s = cpool.tile([N_CHARS, CHAR_EMB], bf16, name="ctab_s")
            nc.sync.dma_start(out=ctab_s[:], in_=ctab[:])
            gidx_s = cpool.tile([128, WPC // 128], mybir.dt.int32, name="gidx_s")
            nc.sync.dma_start(out=gidx_s[:], in_=gidx[:])

            # GloVe gather: 8 blocks of 128 words.
            for j in range(WPC // 128):
                gv = spool.tile([128, WORD_EMB], f32, tag="gv", name="gv")
                nc.gpsimd.indirect_dma_start(
                    out=gv[:],
                    out_offset=None,
                    in_=glove[:],
                    in_offset=bass.IndirectOffsetOnAxis(ap=gidx_s[:, j:j + 1], axis=0),
                )
                nc.sync.dma_start(out=gout_d[j * 128:(j + 1) * 128, :], in_=gv[:])

            for c in range(NCHUNK):
                # ids broadcast across the 102 char-value partitions
                idsb = wpool.tile([N_CHARS, NI], bf16, tag="idsb", name="idsb")
                nc.sync.dma_start(out=idsb[:],
                                  in_=cids[c:c + 1, :].to_broadcast([N_CHARS, NI]))
                # one-hot (in place): OHT[v, pos] = (ids[pos] == v)
                oht = idsb
                nc.vector.tensor_scalar(out=oht[:], in0=idsb[:],
                                        scalar1=iota_s[0:N_CHARS, 0:1],
                                        scalar2=None,
                                        op0=mybir.AluOpType.is_equal)
                # X_T[c, pos] = char_emb[ids[pos], c] via PE + ScalarE evict
                xt3 = wpool.tile([128, NI], bf16, tag="xt3", name="xt3")
                for q in range(NI // 512 + 1):
                    q0 = 512 * q
                    qn = min(512, NI - q0)
                    if qn <= 0:
                        break
                    xps = xpspool.tile([CHAR_EMB, 512], f32, tag="xps", name="xps")
                    nc.tensor.matmul(out=xps[:, 0:qn],
                                     lhsT=ctab_s[:],
                                     rhs=oht[:, q0:q0 + qn],
                                     start=True, stop=True)
                    nc.scalar.copy(out=xt3[0:CHAR_EMB, q0:q0 + qn],
                                   in_=xps[:, 0:qn])
                # im2col shifted copies (partition-crossing => DMA)
                nc.sync.dma_start(out=xt3[30:60, 0:POS_C],
                                  in_=xt3[0:30, 1:POS_C + 1])
                nc.sync.dma_start(out=xt3[60:90, 0:POS_C],
                                  in_=xt3[0:30, 2:POS_C + 2])

                xw = xt3[0:90, 0:POS_C].rearrange("p (w t) -> p w t", t=WLEN)

                for m in range(8):
                    mo, msz = int(CHOFF[m]), CHT[m]
                    pooled_t = spool.tile([128, CHUNK_W], f32, tag="pooled",
                                          name="pooled_t")
                    # 5/8 of (chunk, chtile) units pool via ScalarE-evict
                    # (bf16) + DVE 2x overlap-halving max tree; the rest via
                    # direct DVE reduce from PSUM.  Balances DVE vs ScalarE.
                    use_tree = ((c * 8 + m) % 8) < 5
                    if use_tree:
                        ev = wpool.tile([128, CHUNK_W * TVALID], bf16, tag="ev",
                                        name="ev")
                    w0 = 0
                    mm_i = 0
                    while mm_i < len(MM_WORDS):
                        ps = cpspool.tile([128, 1024], f32, tag="cps", name="ps")
                        nwl = []
                        for half in range(2):
                            if mm_i >= len(MM_WORDS):
                                break
                            nw = MM_WORDS[mm_i]
                            nc.tensor.matmul(
                                out=ps[0:msz, 512 * half:512 * half + nw * TVALID],
                                lhsT=w2_s[:, mo:mo + msz],
                                rhs=xw[:, w0 + sum(nwl):w0 + sum(nwl) + nw, 0:TVALID],
                                start=True, stop=True,
                            )
                            nwl.append(nw)
                            mm_i += 1
                        if use_tree:
                            off = 0
                            for half, nw in enumerate(nwl):
                                e0 = (w0 + off) * TVALID
                                nc.scalar.copy(
                                    out=ev[0:msz, e0:e0 + nw * TVALID],
                                    in_=ps[0:msz,
                                           512 * half:512 * half + nw * TVALID])
                                off += nw
                        elif len(nwl) == 2 and nwl[0] == nwl[1]:
                            rin = (
                                ps[0:msz, :]
                                .rearrange("p (b s) -> p b s", s=512)
                                [:, :, 0:nwl[0] * TVALID]
                                .rearrange("p b (w t) -> p b w t", t=TVALID)
                            )
                            nc.vector.reduce_max(
                                out=pooled_t[0:msz, w0:w0 + 2 * nwl[0]],
                                in_=rin, axis=mybir.AxisListType.X)
                        else:
                            off = 0
                            for half, nw in enumerate(nwl):
                                rin = (
                                    ps[0:msz, 512 * half:512 * half + nw * TVALID]
                                    .rearrange("p (w t) -> p w t", t=TVALID)
                                )
                                nc.vector.reduce_max(
                                    out=pooled_t[0:msz, w0 + off:w0 + off + nw],
                                    in_=rin, axis=mybir.AxisListType.X)
                                off += nw
                        w0 += sum(nwl)
                    if use_tree:
                        # DVE bf16 max tree; widths chosen so every level
                        # keeps even counts and 4B-aligned starts (2x mode).
                        # Overlapping halves are fine: max is idempotent.
                        W = CHUNK_W
                        src = ev[0:msz, :].rearrange("p (w t) -> p w t", t=TVALID)
                        width = TVALID
                        lvl = 0
                        while width > 1:
                            k = (width + 1) // 2
                            if k % 2 == 1 and k > 1:
                                k += 1          # keep counts even for 2x mode
                            if k > 1:
                                dst = spool.tile([128, CHUNK_W * k], bf16,
                                                 tag=f"tr{lvl}", name=f"tr{lvl}",
                                                 bufs=1)
                                dview = dst[0:msz, :].rearrange(
                                    "p (w t) -> p w t", t=k)
                            else:
                                dview = pooled_t[0:msz, :].rearrange(
                                    "p (w t) -> p w t", t=1)
                            nc.vector.tensor_tensor(
                                out=dview,
                                in0=src[:, :, 0:k],
                                in1=src[:, :, width - k:width],
                                op=mybir.AluOpType.max)
                            if k > 1:
                                src = dst[0:msz, :].rearrange(
                                    "p (w t) -> p w t", t=k)
                            width = k
                            lvl += 1
                    # bias add on ScalarE (out = in*1 + bias[p])
                    nc.scalar.add(out=pooled_t[0:msz, :], in_=pooled_t[0:msz, :],
                                  add=bias_s[0:msz, m:m + 1])
                    nc.sync.dma_start(
                        out=pooled_d[mo:mo + msz, c * CHUNK_W:(c + 1) * CHUNK_W],
                        in_=pooled_t[0:msz, :],
                    )
    return nc


# ---------------------------------------------------------------- host prep
def _prep_shared(char_emb, conv_w, conv_b, glove):
    ctab = char_emb.astype(ml_dtypes.bfloat16)           # [102, 30]

    w2 = np.zeros((90, NCH), np.float32)
    o = np.arange(NCH)
    g = o // N_FILT
    for k in range(KSIZE):
        w2[k * CHAR_EMB + g, o] = conv_w[o, 0, k]
    w2 = w2.astype(ml_dtypes.bfloat16)

    biasp = np.zeros((128, 8), np.float32)
    for m in range(8):
        biasp[:CHT[m], m] = conv_b[CHOFF[m]:CHOFF[m] + CHT[m]]

    iota = np.arange(128, dtype=np.float32).reshape(128, 1)
    glove = np.ascontiguousarray(glove.astype(np.float32))
    return ctab, w2, biasp, iota, glove


def _prep_core(core, char_ids_flat_pad, word_ids_flat):
    cids = np.zeros((NCHUNK, NI), np.float32)
    base = core * WPC * WLEN
    for c in range(NCHUNK):
        cids[c, :POS_C + 2] = char_ids_flat_pad[base + c * POS_C:
                                                base + c * POS_C + POS_C + 2]
    cids = cids.astype(ml_dtypes.bfloat16)
    wseg = word_ids_flat[core * WPC:(core + 1) * WPC].astype(np.int32)
    gidx = wseg.reshape(WPC // 128, 128).T.copy()        # [128, 8]
    return cids, gidx


def kernel(char_ids, word_ids, char_emb, conv_w, conv_b, glove):
    char_ids = np.asarray(char_ids)
    word_ids = np.asarray(word_ids)
    char_emb = np.asarray(char_emb, np.float32)
    conv_w = np.asarray(conv_w, np.float32)
    conv_b = np.asarray(conv_b, np.float32)
    glove = np.asarray(glove, np.float32)

    if "nc" not in _CACHE:
        nc = build_program()
        if not nc.is_finalized():
            nc.finalize()   # Bacc compile passes (register alloc, lib loads)
        _CACHE["nc"] = nc
    nc = _CACHE["nc"]

    ctab, w2, biasp, iota, glove_c = _prep_shared(char_emb, conv_w, conv_b, glove)

    ids_flat_pad = np.concatenate(
        [char_ids.reshape(-1), np.zeros(2, np.int64)]).astype(np.int64)
    word_ids_flat = word_ids.reshape(-1)

    in_maps = []
    for core in range(NCORES):
        cids, gidx = _prep_core(core, ids_flat_pad, word_ids_flat)
        in_maps.append({
            "ctab": ctab,
            "w2": w2,
            "biasp": biasp,
            "iota": iota,
            "cids": cids,
            "gidx": gidx,
            "glove": glove_c,
        })

    import os
    trace = bool(int(os.environ.get("BASS_KERNEL_TRACE", "0")))
    res = run_bass_kernel_spmd(nc, in_maps, core_ids=list(range(NCORES)),
                               trace=trace)
    _CACHE["last_result"] = res
    results = res.results

    out = np.empty((WORDS, NCH + WORD_EMB), np.float32)
    for core in range(NCORES):
        r = results[core]
        out[core * WPC:(core + 1) * WPC, :NCH] = r["pooled"].T
        out[core * WPC:(core + 1) * WPC, NCH:] = r["gout"]
    return out.reshape(B, S, NCH + WORD_EMB)
